# revision 1
# baseline (speedup 1.0000x reference)
"""Trainium2 Bass kernel for nn_Discriminator (MLP + BN + attn + minibatch discrimination).

Strategy (8 NeuronCores, no collectives):
  - Shard the O(B^2) MBD block over the output index j: core d computes scores for
    batch rows [128d, 128d+128). SPMD programs are identical; the shard is selected
    by giving core d a batch-rolled copy of x (np.roll by -128d), so "my j's" are
    always local rows 0..127 while the i-sum still runs over the full batch.
  - The attention block and T projection are folded on the host:
      G = I + Wv@Wo ; M = h3 @ (G@T2) + (bv@Wo+bo)@T2 ; score_h = (G@Ws_h)^T h3 + c
    so the device MLP is just W1 -> BN -> W3 -> M, all matmuls in bf16 with
    biases injected via ones-row matmuls (single leaky-relu epilogue per chunk).
  - Pairwise block per j: |d| = 2relu(d) - d; one dual-op tensor_scalar per
    125-row fk-tile computes A = relu(M^T - M^T[:, j]) (bf16); 0/1 selection
    matmuls on the PE sum over k (5) into PSUM (two j's packed at PSUM
    partition offsets 0 and 64 via col-tiling), a full-width matmul adds the
    -0.5*sum_k M_i correction, and one activation(Exp, scale=-2, bias=-S_j,
    accum_out=...) computes exp(-d) and the i-sum in a single ACT op. One of
    the four A-ops per pair runs on the (otherwise idle) Pool engine to
    offload the DVE bottleneck.
  - score = Wsh'.T h3 + Ws_o.T o + (bs - sum(Ws_o)), biases folded on host.
"""

import numpy as np
from contextlib import ExitStack

import ml_dtypes
import concourse.bass as bass
import concourse.tile as tile
from concourse import mybir
from concourse.bass_utils import run_bass_kernel_spmd

F32 = mybir.dt.float32
BF16 = mybir.dt.bfloat16
AF = mybir.ActivationFunctionType
ALU = mybir.AluOpType
AX = mybir.AxisListType

B = 1024
IN_DIM = 128
NCORES = 8
JSH = B // NCORES          # 128 j's per core
NPAIR = JSH // 2           # 64 pairs of j's
NF = 50
BN_EPS = 1e-5
POOL_A = False             # offload 1 of 4 per-pair A-ops to the Pool engine

# CPB (bf16) column layout
_C_W1 = 0          # [128, 256]
_C_W2 = 256        # [128, 256] (two k-tiles of W2)
_C_W3 = 512        # [128, 64]
_C_SA = 576        # [125, 64]
_C_SB = 640        # [125, 64]
_C_WT = 704        # [65, 250]  G@T2 with bias row
_C_WSH = 954       # [65, 1]    G@Ws_h with bias row
_C_B1 = 955        # [1, 256]
_C_B3 = 1211       # [1, 64]
_C_ONES = 1275     # [1, 512]
_C_WTS = 1787      # [65, 50]   -sum_k WT[:, 5f+k] (with bias row)
_C_I50 = 1837      # [50, 128]  0.5 on diag at cols 0:50 and 64:114
_C_END = 1965

_CACHE: dict = {}


def _emit_body(tc, d, score_out):
    nc = tc.nc
    ctx = ExitStack()
    with ctx:
        consts = ctx.enter_context(tc.tile_pool(name="consts", bufs=1))
        mlp = ctx.enter_context(tc.tile_pool(name="mlp", bufs=1))
        small = ctx.enter_context(tc.tile_pool(name="small", bufs=1))

        CPB = consts.tile([128, _C_END], BF16, tag="CPB")
        CPF = consts.tile([128, 8], F32, tag="CPF")
        xTb = mlp.tile([128, B], BF16, tag="xTb")
        # ring split: W1/W2/W3 head + CPF on scalar, x on sync, the rest on gpsimd
        nc.scalar.dma_start(CPB[:, 0:_C_SA], d["CPB"][:, 0:_C_SA])
        nc.scalar.dma_start(CPF[:], d["CPF"][:])
        nc.sync.dma_start(xTb[:, 0:512], d["xTb"][:, 0:512])
        nc.sync.dma_start(xTb[:, 512:B], d["xTb"][:, 512:B])
        nc.gpsimd.dma_start(CPB[:, _C_SA:_C_END], d["CPB"][:, _C_SA:_C_END])
        # touch Exp early so the activation-table load runs off the critical path
        SM = small.tile([128, 148], F32, tag="SM")
        nc.vector.memset(SM[0:1, 0:1], 0.0)
        nc.scalar.activation(SM[0:1, 1:2], SM[0:1, 0:1], AF.Exp, bias=0.0, scale=1.0)

        W1 = CPB[:, _C_W1:_C_W1 + 256]
        W2 = CPB[:, _C_W2:_C_W2 + 256]
        W3 = CPB[:, _C_W3:_C_W3 + 64]
        Sa = CPB[0:125, _C_SA:_C_SA + 64]
        Sb = CPB[0:125, _C_SB:_C_SB + 64]
        WT = CPB[0:65, _C_WT:_C_WT + 250]
        WSH = CPB[0:65, _C_WSH:_C_WSH + 1]
        WTS = CPB[0:65, _C_WTS:_C_WTS + 50]
        I50h2 = CPB[0:50, _C_I50:_C_I50 + 128]
        gamma = CPF[:, 0:1]
        beta = CPF[:, 1:2]
        WsO = CPF[0:50, 2:3]
        bsf = CPF[0:1, 3:4]
        b1a = CPF[:, 4:5]
        b1b = CPF[:, 5:6]
        b3 = CPF[0:64, 6:7]

        # ---- persistent activations ----
        h1T = mlp.tile([128, 2 * B], BF16, tag="h1T")     # [256,1024] as 2 m-tiles
        hbnT = mlp.tile([128, B], BF16, tag="hbnT")
        h3T = mlp.tile([65, B], BF16, tag="h3T")          # row 64 = ones (bias row)
        MTb = mlp.tile([125, 2 * B], BF16, tag="MTb")     # [250,1024] as 2 fk-tiles
        MTf = mlp.tile([125, 2 * JSH], F32, tag="MTf")    # f32 scalars, local j's only
        SMTnb = mlp.tile([50, B], BF16, tag="SMTnb")      # -sum_k M_i, bf16 full i
        SMTnl = mlp.tile([50, JSH], F32, tag="SMTnl")     # -sum_k M_j, local j's
        BIASP = mlp.tile([128, NPAIR], F32, tag="BIASP")
        OBUF = mlp.tile([128, NPAIR], F32, tag="OBUF")
        O50 = mlp.tile([50, NPAIR, 2], F32, tag="O50")

        def lrelu(dst, src):
            # dst = max(src, 0.2*src)
            nc.vector.scalar_tensor_tensor(
                out=dst, in0=src, scalar=0.2, in1=src, op0=ALU.mult, op1=ALU.max
            )

        nc.vector.memset(h3T[64:65, :], 1.0)
        nc.vector.memset(BIASP[:], 0.0)

        with tc.tile_pool(name="ph1_psum", bufs=1, space=bass.MemorySpace.PSUM) as pp, \
             tc.tile_pool(name="ph1_sb", bufs=2) as sb:
            # ---- h1T = lrelu(W1.T xTb + b1) ----
            for mt, b1t in ((0, b1a), (1, b1b)):
                for c in range(2):
                    cs = slice(512 * c, 512 * (c + 1))
                    ps = pp.tile([128, 512], F32, tag="ps", bufs=2)
                    nc.tensor.matmul(ps[:], W1[:, 128 * mt:128 * (mt + 1)],
                                     xTb[:, cs], start=True, stop=True)
                    tt = sb.tile([128, 512], BF16, tag="tt")
                    nc.scalar.activation(tt[:], ps[:], AF.Identity, bias=b1t, scale=1.0)
                    lrelu(h1T[:, B * mt + 512 * c: B * mt + 512 * (c + 1)], tt[:])

            # ---- h2 (kept in PSUM) + BN stats ----
            # b2 folds into BN shift: (h+b2) - mean(h+b2) = h - mean(h), so skip it.
            h2ps = []
            sums = SM[:, 2:6]   # per-chunk sum, sumsq
            for c in range(2):
                cs = slice(512 * c, 512 * (c + 1))
                ps = pp.tile([128, 512], F32, tag=f"h2ps{c}")
                for kt in range(2):
                    nc.tensor.matmul(ps[:], W2[:, 128 * kt:128 * (kt + 1)],
                                     h1T[:, B * kt + 512 * c: B * kt + 512 * (c + 1)],
                                     start=(kt == 0), stop=(kt == 1))
                nc.vector.tensor_reduce(sums[:, c:c + 1], ps[:], axis=AX.X, op=ALU.add)
                sq = sb.tile([128, 512], F32, tag="sq")
                nc.scalar.activation(sq[:], ps[:], AF.Square, bias=0.0, scale=1.0,
                                     accum_out=sums[:, 2 + c:3 + c])
                h2ps.append(ps)

            # mu = (s0+s1)/1024 ; msq = (q0+q1)/1024 ; var = msq - mu^2
            mu = SM[:, 6:7]
            nc.vector.scalar_tensor_tensor(out=mu[:], in0=sums[:, 0:1], scalar=1.0 / B,
                                           in1=sums[:, 1:2], op0=ALU.bypass, op1=ALU.add)
            nc.vector.tensor_scalar(out=mu[:], in0=mu[:], scalar1=1.0 / B, scalar2=None,
                                    op0=ALU.mult)
            msq = SM[:, 7:8]
            nc.vector.scalar_tensor_tensor(out=msq[:], in0=sums[:, 2:3], scalar=1.0,
                                           in1=sums[:, 3:4], op0=ALU.bypass, op1=ALU.add)
            nc.vector.tensor_scalar(out=msq[:], in0=msq[:], scalar1=1.0 / B, scalar2=None,
                                    op0=ALU.mult)
            var = SM[:, 8:9]
            nc.vector.scalar_tensor_tensor(out=var[:], in0=mu[:], scalar=-1.0,
                                           in1=mu[:], op0=ALU.mult, op1=ALU.mult)
            nc.vector.tensor_tensor(out=var[:], in0=var[:], in1=msq[:], op=ALU.add)
            # invstd = exp(-0.5*ln(var+eps))  (avoids the banned Rsqrt + table swap)
            eps_t = SM[:, 9:10]
            nc.vector.memset(eps_t[:], BN_EPS)
            lnv = SM[:, 10:11]
            nc.scalar.activation(lnv[:], var[:], AF.Ln, bias=eps_t[:], scale=1.0)
            invstd = SM[:, 11:12]
            nc.scalar.activation(invstd[:], lnv[:], AF.Exp, bias=0.0, scale=-0.5)
            # s = gamma*invstd ; bb = beta - mu*s
            s = SM[:, 12:13]
            nc.vector.tensor_tensor(out=s[:], in0=invstd[:], in1=gamma[:], op=ALU.mult)
            bb = SM[:, 13:14]
            nc.vector.scalar_tensor_tensor(out=bb[:], in0=mu[:], scalar=-1.0,
                                           in1=s[:], op0=ALU.mult, op1=ALU.mult)
            nc.vector.tensor_tensor(out=bb[:], in0=bb[:], in1=beta[:], op=ALU.add)

            # hbnT = lrelu(s*h2 + bb)   (ACT applies affine -> bf16, DVE lrelu in 4x mode)
            for c in range(2):
                tt = sb.tile([128, 512], BF16, tag="tt")
                nc.scalar.activation(tt[:], h2ps[c][:], AF.Identity, bias=bb[:, 0:1],
                                     scale=s[:, 0:1])
                lrelu(hbnT[:, 512 * c:512 * (c + 1)], tt[:])

            # ---- h3T = lrelu(W3.T hbnT + b3) ----
            for c in range(2):
                cs = slice(512 * c, 512 * (c + 1))
                ps = pp.tile([64, 512], F32, tag="ps64", bufs=1)
                nc.tensor.matmul(ps[:], W3[:], hbnT[:, cs], start=True, stop=True)
                tt = sb.tile([64, 512], BF16, tag="tt64")
                nc.scalar.activation(tt[:], ps[:], AF.Identity, bias=b3, scale=1.0)
                lrelu(h3T[0:64, cs], tt[:])

            # ---- SMTn[f, i] = -sum_k M[i, 5f+k]  (for the |d|=2relu(d)-d trick) ----
            for c in range(2):
                ps = pp.tile([50, 512], F32, tag="psm2", bufs=1)
                nc.tensor.matmul(ps[:], WTS[:], h3T[:, 512 * c:512 * (c + 1)],
                                 start=True, stop=True)
                nc.scalar.activation(SMTnb[:, 512 * c:512 * (c + 1)], ps[:], AF.Copy,
                                     bias=0.0, scale=1.0)
                if c == 0:
                    nc.vector.tensor_copy(SMTnl[:], ps[:, 0:JSH])

            # per-pair exp bias rows: [0:50] <- SMTn col j1, [64:114] <- SMTn col j2
            nc.vector.tensor_copy(BIASP[0:50, :], SMTnl[:].rearrange(
                "p (a b) -> p a b", b=2)[:, :, 0:1])
            nc.vector.tensor_copy(BIASP[64:114, :], SMTnl[:].rearrange(
                "p (a b) -> p a b", b=2)[:, :, 1:2])

            # ---- MT = WT.T h3T  ([250,1024] as 2 fk-tiles), bf16 + f32 j-scalars ----
            for st in range(2):
                for c in range(2):
                    ps = pp.tile([125, 512], F32, tag="psm", bufs=2)
                    nc.tensor.matmul(ps[:], WT[:, 125 * st:125 * (st + 1)],
                                     h3T[:, 512 * c:512 * (c + 1)], start=True, stop=True)
                    sl = slice(B * st + 512 * c, B * st + 512 * (c + 1))
                    nc.scalar.activation(MTb[:, sl], ps[:], AF.Copy, bias=0.0, scale=1.0)
                    if c == 0:
                        nc.vector.tensor_copy(MTf[:, JSH * st:JSH * (st + 1)], ps[:, 0:JSH])


        # ---- pairwise MBD block ----
        # d[f,i] for row j is sum_k |M_i - M_j| = 2*sum_k relu(M_i - M_j)
        #   - sum_k M_i + sum_k M_j.  PSUM accumulates P = S@A + 0.5*SMTn_i;
        # exp(-d) = Exp(-2*P + bias) with per-partition bias = SMTn[:, j].
        with tc.tile_pool(name="apool", bufs=4) as apool, \
             tc.tile_pool(name="dpool", bufs=3, space=bass.MemorySpace.PSUM) as dpool, \
             tc.tile_pool(name="spsum", bufs=1, space=bass.MemorySpace.PSUM) as sp:
            ssum = sp.tile([1, JSH], F32, tag="ssum")
            nc.tensor.matmul(ssum[:], WSH[:], h3T[:, 0:JSH], start=True, stop=False,
                             skip_group_check=True)
            for jp in range(NPAIR):
                j1, j2 = 2 * jp, 2 * jp + 1
                As = {}
                for (jj, col) in ((j1, 0), (j2, 64)):
                    for st in range(2):
                        A = apool.tile([125, B], BF16, tag=f"A{col}{st}")
                        eng = nc.gpsimd if (POOL_A and col == 64 and st == 1) \
                            else nc.vector
                        eng.tensor_scalar(
                            out=A[:], in0=MTb[:, B * st:B * (st + 1)],
                            scalar1=MTf[:, JSH * st + jj:JSH * st + jj + 1],
                            scalar2=0.0, op0=ALU.subtract, op1=ALU.max)
                        As[(col, st)] = A
                dps = dpool.tile([128, B], F32, tag="dps")
                for st, S in ((0, Sa), (1, Sb)):
                    for c in range(2):
                        cs = slice(512 * c, 512 * (c + 1))
                        for col in (0, 64):
                            nc.tensor.matmul(dps[col:col + 64, cs], S[:],
                                             As[(col, st)][:, cs],
                                             start=(st == 0), stop=False,
                                             tile_position=(0, col),
                                             skip_group_check=True)
                for c in range(2):
                    cs = slice(512 * c, 512 * (c + 1))
                    nc.tensor.matmul(dps[0:128, cs], I50h2[:], SMTnb[:, cs],
                                     start=False, stop=True,
                                     skip_group_check=True)
                nc.scalar.activation(dps[0:114, :], dps[0:114, :], AF.Exp,
                                     bias=BIASP[0:114, jp:jp + 1], scale=-2.0,
                                     accum_out=OBUF[0:114, jp:jp + 1])

            # ---- o columns -> j-ordered [50, 128] ----
            nc.vector.tensor_copy(O50[:, :, 0:1], OBUF[0:50, :])
            nc.vector.tensor_copy(O50[:, :, 1:2], OBUF[64:114, :])

            # ---- score = WSH.T h3T[:, :128] (done above) + WsO.T O + bsf ----
            nc.tensor.matmul(ssum[:], WsO[:], O50[:, :, :], start=False, stop=True,
                             skip_group_check=True)
            sc = SM[0:1, 20:20 + JSH]
            nc.scalar.activation(sc[:], ssum[:], AF.Identity, bias=bsf[0:1, 0:1],
                                 scale=1.0)
            nc.gpsimd.dma_start(score_out[:], sc[:])


def _split_waits(nc):
    """Hoist excess semaphore waits onto single-wait engine nops.

    This walrus build's codegen rejects instructions whose ISA struct carries
    more than one sync-wait ("Too many sync wait commands"). Engine instruction
    streams execute in order, so moving all waits of an instruction onto nop
    instructions spliced immediately before it (one wait per nop, same engine)
    is semantically identical. DMA instructions are left untouched (their waits
    ride the DGE descriptor, not the engine stream) and are asserted to have
    <=1 wait.
    """
    from concourse import mybir as mb
    DMA_TYPES = (mb.InstDMACopy, mb.InstDMA, mb.InstTriggeredCopy) \
        if hasattr(mb, "InstTriggeredCopy") else (mb.InstDMACopy, mb.InstDMA)
    for fn in nc.m.functions:
        for bb in fn.blocks:
            insts = list(bb.instructions)
            out = []
            for inst in insts:
                si = inst.sync_info
                waits = list(si.on_wait) if si is not None else []
                if len(waits) > 1:
                    if isinstance(inst, DMA_TYPES):
                        raise AssertionError(
                            f"DMA instruction {inst.name} has {len(waits)} waits; "
                            "cannot split safely — restructure the kernel")
                    for w in waits:
                        nop = mb.InstNoOp(
                            name=nc.get_next_instruction_name(),
                            ins=[], outs=[])
                        nop.engine = inst.engine
                        nop.sync_info = mb.SyncInfo(on_wait=[w], on_update=[])
                        nc.register_instruction(nop)
                        out.append(nop)
                    inst.sync_info = mb.SyncInfo(
                        on_wait=[], on_update=list(si.on_update))
                out.append(inst)
            bb.instructions = out


def _build():
    nc = bass.Bass("TRN2", target_bir_lowering=False, debug=False,
                   num_devices=NCORES)
    d = {}

    def din(name, shape, dtype=F32):
        d[name] = nc.dram_tensor(name, shape, dtype, kind="ExternalInput").ap()

    din("xTb", [IN_DIM, B], BF16)
    din("CPB", [128, _C_END], BF16)
    din("CPF", [128, 8])
    score = nc.dram_tensor("score", [1, JSH], F32, kind="ExternalOutput").ap()

    with tile.TileContext(nc) as tc:
        _emit_body(tc, d, score)
    _split_waits(nc)
    return nc


def get_nc():
    if "nc" not in _CACHE:
        _CACHE["nc"] = _build()
    return _CACHE["nc"]


def _make_in_maps(inputs):
    f = lambda a: np.ascontiguousarray(np.asarray(a, dtype=np.float32))
    x = f(inputs["x"])
    W1 = f(inputs["W1"])            # [128, 256]
    W2 = f(inputs["W2"])            # [256, 128]
    W3 = f(inputs["W3"])            # [128, 64]
    Wv, bv = f(inputs["Wv"]), f(inputs["bv"]).reshape(-1)
    Wo, bo = f(inputs["Wo"]), f(inputs["bo"]).reshape(-1)
    T2 = f(inputs["T"]).reshape(64, 250)
    Ws = f(inputs["Ws"])            # [114, 1]
    bs = float(f(inputs["bs"]).reshape(-1)[0])

    # fold attention: h' = h3 @ G + g ; M = h' @ T2 ; score_h = h' @ Ws_h
    G = np.eye(64, dtype=np.float32) + Wv @ Wo          # [64, 64]
    g = bv @ Wo + bo                                    # [64]
    WT = np.zeros((65, 250), np.float32)
    WT[0:64] = G @ T2
    WT[64] = g @ T2
    WSH = np.zeros((65, 1), np.float32)
    WSH[0:64] = G @ Ws[0:64]
    WSH[64, 0] = float(g @ Ws[0:64, 0])

    Sa = np.zeros((125, 64), np.float32)
    Sb = np.zeros((125, 64), np.float32)
    for fk in range(125):
        Sa[fk, fk // 5] = 1.0
        Sb[fk, 25 + fk // 5] = 1.0
    WTS = -WT.reshape(65, 50, 5).sum(axis=2)            # [65, 50]
    I50h2 = np.zeros((50, 128), np.float32)
    np.fill_diagonal(I50h2[:, 0:50], 0.5)
    np.fill_diagonal(I50h2[:, 64:114], 0.5)

    CPB = np.zeros((128, _C_END), np.float32)
    CPB[:, _C_W1:_C_W1 + 256] = W1
    CPB[:, _C_W2:_C_W2 + 128] = W2[0:128]
    CPB[:, _C_W2 + 128:_C_W2 + 256] = W2[128:256]
    CPB[:, _C_W3:_C_W3 + 64] = W3
    CPB[0:125, _C_SA:_C_SA + 64] = Sa
    CPB[0:125, _C_SB:_C_SB + 64] = Sb
    CPB[0:65, _C_WT:_C_WT + 250] = WT
    CPB[0:65, _C_WSH:_C_WSH + 1] = WSH
    CPB[0, _C_B1:_C_B1 + 256] = f(inputs["b1"]).reshape(-1)
    CPB[0, _C_B3:_C_B3 + 64] = f(inputs["b3"]).reshape(-1)
    CPB[0, _C_ONES:_C_ONES + 512] = 1.0
    CPB[0:65, _C_WTS:_C_WTS + 50] = WTS
    CPB[0:50, _C_I50:_C_I50 + 128] = I50h2

    CPF = np.zeros((128, 8), np.float32)
    CPF[:, 0] = f(inputs["gamma"]).reshape(-1)
    CPF[:, 1] = f(inputs["beta"]).reshape(-1)
    CPF[0:50, 2] = Ws[64:114, 0]
    CPF[0, 3] = bs - float(Ws[64:114].sum())
    CPF[:, 4] = f(inputs["b1"]).reshape(-1)[0:128]
    CPF[:, 5] = f(inputs["b1"]).reshape(-1)[128:256]
    CPF[0:64, 6] = f(inputs["b3"]).reshape(-1)

    common = {
        "CPB": CPB.astype(ml_dtypes.bfloat16),
        "CPF": CPF,
    }
    in_maps = []
    for c in range(NCORES):
        m = dict(common)
        m["xTb"] = np.ascontiguousarray(
            np.roll(x, -JSH * c, axis=0).T.astype(ml_dtypes.bfloat16))
        in_maps.append(m)
    return in_maps


def kernel(**inputs) -> np.ndarray:
    nc = get_nc()
    in_maps = _make_in_maps(inputs)
    res = run_bass_kernel_spmd(nc, in_maps, list(range(NCORES)))
    outs = [np.asarray(res.results[c]["score"]).reshape(JSH) for c in range(NCORES)]
    return np.concatenate(outs).astype(np.float32)


if __name__ == "__main__":
    print("building nc...")
    nc = get_nc()
    print("build OK")

